# revision 1
# baseline (speedup 1.0000x reference)
"""MoE transformer MLP (top-2 of 8 experts) + log_softmax head, on 8 trn2 cores.

Sharding: data-parallel over the batch dim — core c owns batch row c
(1024 tokens) end-to-end, with all expert weights replicated. The second
GEMM (h @ w2) is algebraically folded: the model output is
log_softmax_S(sum_d y), and sum_d (h @ w2[e] + b2[e]) = h . (w2[e] @ 1) +
sum_d b2[e], so each core only needs w2sum[e] = w2[e].sum(-1) (computed
on-device) and never materializes the [T, D] expert outputs. No
collectives: each core returns its batch row's [1024] log-softmax.
"""

import sys

for _p in ("/opt/trn_rl_repo",):
    if _p not in sys.path:
        sys.path.insert(0, _p)

import numpy as np
import ml_dtypes

B, S, D, H, E = 8, 1024, 512, 2048, 8
TLOC = S          # tokens per core (one batch row)
BLKS = TLOC // 128  # 8 token blocks of 128
KC = D // 128     # 4 contraction chunks
NH = H // 512     # 4 psum-bank-wide slices of H

_CACHE = {}

import os
KCUT = int(os.environ.get("KCUT", "0"))  # 0=full; probes: 5,7,8,1,2,3
DO_SETUP = KCUT != 5
DO_X = KCUT not in (5, 7)
DO_GATE = KCUT not in (5, 7, 8)
DO_ROUTING = KCUT in (0, 2, 3)
DO_MAIN = KCUT in (0, 3)


def _tail(nc, tc, psf, fin, singles, out_d, ident, ones_col, ones_row, y_sb, f32, ALU, ACT, AX):
    yT_ps = psf.tile([BLKS, 128], f32, tag="yT")
    nc.tensor.transpose(yT_ps, y_sb, ident)
    yT_sb = fin.tile([BLKS, 128], f32, tag="yTs")
    nc.vector.tensor_copy(out=yT_sb, in_=yT_ps)
    bmax = fin.tile([BLKS, 1], f32, tag="bmax")
    nc.vector.reduce_max(bmax, yT_sb, axis=AX.X)
    bT_ps = psf.tile([1, BLKS], f32, tag="bT")
    nc.tensor.transpose(bT_ps, bmax, ident[:BLKS, :BLKS])
    brow = fin.tile([1, BLKS], f32, tag="brow")
    nc.vector.tensor_copy(out=brow, in_=bT_ps)
    gmax = fin.tile([1, 1], f32, tag="gmax")
    nc.vector.reduce_max(gmax, brow, axis=AX.X)
    gmax_ps = psf.tile([128, 1], f32, tag="gmaxp")
    nc.tensor.matmul(gmax_ps, ones_row, gmax, start=True, stop=True)
    gmax_bc = fin.tile([128, 1], f32, tag="gmaxb")
    nc.vector.tensor_copy(out=gmax_bc, in_=gmax_ps)
    esb = fin.tile([128, BLKS], f32, tag="esb")
    nc.vector.tensor_scalar(
        out=esb, in0=y_sb, scalar1=gmax_bc, scalar2=None, op0=ALU.subtract)
    ex = fin.tile([128, BLKS], f32, tag="ex")
    rowsum = fin.tile([128, 1], f32, tag="rowsum")
    nc.scalar.activation(out=ex, in_=esb, func=ACT.Exp, accum_out=rowsum)
    tot = psf.tile([1, 1], f32, tag="tot")
    nc.tensor.matmul(tot, ones_col, rowsum, start=True, stop=True)
    lse = fin.tile([1, 1], f32, tag="lse")
    nc.scalar.activation(out=lse, in_=tot, func=ACT.Ln)
    nc.vector.tensor_add(lse, lse, gmax)
    lse_ps = psf.tile([128, 1], f32, tag="lsep")
    nc.tensor.matmul(lse_ps, ones_row, lse, start=True, stop=True)
    lse_bc = fin.tile([128, 1], f32, tag="lseb")
    nc.vector.tensor_copy(out=lse_bc, in_=lse_ps)
    outsb = fin.tile([128, BLKS], f32, tag="outsb")
    nc.vector.tensor_scalar(
        out=outsb, in0=y_sb, scalar1=lse_bc, scalar2=None, op0=ALU.subtract)
    nc.sync.dma_start(
        out=out_d[:].rearrange("(b p) -> p b", p=128), in_=outsb)


def _build(has_b1: bool):
    import concourse.bass as bass  # noqa: F401
    import concourse.tile as tile
    import concourse.mybir as mybir
    from concourse import bacc

    dt = mybir.dt
    f32 = dt.float32
    f32r = dt.float32r
    ALU = mybir.AluOpType
    ACT = mybir.ActivationFunctionType
    AX = mybir.AxisListType

    nc = bacc.Bacc(None, target_bir_lowering=False)

    with tile.TileContext(nc) as tc:
        with tc.tile_pool(name="dram", bufs=1, space="DRAM") as dram:
            x_d = dram.tile([TLOC, D], f32, kind="ExternalInput", name="x_shard", uniquify=False)
            gwt_d = dram.tile([E, D], f32, kind="ExternalInput", name="gate_w_t", uniquify=False)
            gb_d = dram.tile([E], f32, kind="ExternalInput", name="gate_b", uniquify=False)
            w1_d = dram.tile([E, D, H], dt.float16, kind="ExternalInput", name="w1", uniquify=False)
            b1_d = dram.tile([E, H], f32, kind="ExternalInput", name="b1", uniquify=False)
            w2_d = dram.tile([E, H, D], f32, kind="ExternalInput", name="w2", uniquify=False)
            b2_d = dram.tile([E, D], f32, kind="ExternalInput", name="b2", uniquify=False)
            id_d = dram.tile([128, 128], f32, kind="ExternalInput", name="ident128", uniquify=False)
            out_d = dram.tile([TLOC], f32, kind="ExternalOutput", name="out", uniquify=False)
            w2s_d = dram.tile([E, H], f32, name="w2s_scratch")
            b2s_d = dram.tile([E], f32, name="b2s_scratch")

            with tc.tile_pool(name="singles", bufs=1) as singles:
                ident = singles.tile([128, 128], f32)
                nc.sync.dma_start(out=ident, in_=id_d[:])
                ones_col = singles.tile([128, 1], f32)
                nc.vector.memset(ones_col, 1.0)
                ones_row = singles.tile([1, 128], f32)
                nc.vector.memset(ones_row, 1.0)

                if KCUT in (5, 7):
                    dbg = singles.tile([128, BLKS], f32)
                    nc.vector.tensor_copy(out=dbg, in_=ident[:, :BLKS])
                    nc.sync.dma_start(
                        out=out_d[:].rearrange("(b p) -> p b", p=128), in_=dbg)

                # gate weights broadcast along partitions: [128, E, D]
                gw_bc = singles.tile([128, E, D], f32)
                if DO_SETUP:
                    nc.gpsimd.dma_start(
                        out=gw_bc,
                        in_=bass.AP(tensor=gwt_d.tensor, offset=gwt_d.offset,
                                    ap=[[0, 128]] + [list(a) for a in gwt_d.ap]),
                    )
                gb_bc = singles.tile([128, E], f32)
                if DO_SETUP:
                    nc.gpsimd.dma_start(
                        out=gb_bc,
                        in_=bass.AP(tensor=gb_d.tensor, offset=gb_d.offset,
                                    ap=[[0, 128]] + [list(a) for a in gb_d.ap]),
                    )
                if has_b1:
                    b1_sb = singles.tile([1, E, H], f32)
                    nc.sync.dma_start(out=b1_sb, in_=b1_d[None])

                # b2sum[e] = sum_d b2[e, d], broadcast to [128, E]
                b2s_bc = singles.tile([128, E], f32)
                if DO_SETUP:
                    b2_sb = singles.tile([E, D], f32)
                    nc.sync.dma_start(out=b2_sb, in_=b2_d[:])
                    b2s_sb = singles.tile([E, 1], f32)
                    nc.vector.reduce_sum(b2s_sb, b2_sb, axis=AX.X)
                    nc.sync.dma_start(out=b2s_d[:, None], in_=b2s_sb)
                    nc.gpsimd.dma_start(
                        out=b2s_bc,
                        in_=bass.AP(tensor=b2s_d.tensor, offset=b2s_d.offset,
                                    ap=[[0, 128]] + [list(a) for a in b2s_d.ap]),
                    )

                # xT: [128d, kc, TLOC] via PE transposes of x blocks (fp32r-rounded)
                if DO_X:
                    xT = singles.tile([128, KC, TLOC], dt.float16)
                    logits = singles.tile([128, BLKS, E], f32)
                    eq1 = singles.tile([128, BLKS, E], f32)
                    eq2 = singles.tile([128, BLKS, E], f32)
                    w_tok = singles.tile([128, BLKS, E], f32)
                    dm_all = singles.tile([128, BLKS], f32)
                    s2_all = singles.tile([128, BLKS], f32)
                    s1_all = singles.tile([128, BLKS], f32)
                    phat = singles.tile([128, BLKS, E], f32)
                    y_sb = singles.tile([128, BLKS], f32)

                with tc.tile_pool(name="xload", bufs=3) as xload, \
                     tc.tile_pool(name="pst", bufs=4, space="PSUM") as pst, \
                     tc.tile_pool(name="gsc", bufs=4) as gsc, \
                     tc.tile_pool(name="rt", bufs=4) as rt:
                    for blk in range(BLKS if DO_X else 0):
                        x_sb = xload.tile([128, D], f32, tag="x")
                        nc.sync.dma_start(out=x_sb, in_=x_d[blk * 128:(blk + 1) * 128, :])
                        for k in range(KC):
                            tp = pst.tile([128, 128], f32, tag="tp")
                            nc.tensor.transpose(tp, x_sb[:, k * 128:(k + 1) * 128], ident)
                            nc.vector.tensor_copy(
                                out=xT[:, k, blk * 128:(blk + 1) * 128], in_=tp)
                        # gate logits on DVE (full fp32): sum_d x * gate_w[e, :]
                        for e in range(E if DO_GATE else 0):
                            scr = gsc.tile([128, D], f32, tag="scr")
                            nc.vector.tensor_mul(scr, x_sb, gw_bc[:, e, :])
                            nc.vector.reduce_sum(
                                logits[:, blk, e:e + 1], scr, axis=AX.X)
                        if DO_GATE:
                            nc.vector.tensor_add(
                                logits[:, blk, :], logits[:, blk, :], gb_bc)

                    if KCUT == 8:
                        dbg = singles.tile([128, BLKS], f32)
                        nc.vector.tensor_copy(out=dbg, in_=xT[:, 0, :BLKS].bitcast(f32))
                        nc.sync.dma_start(
                            out=out_d[:].rearrange("(b p) -> p b", p=128), in_=dbg)
                    if KCUT == 1:
                        dbg = singles.tile([128, BLKS], f32)
                        nc.vector.tensor_copy(out=dbg, in_=logits[:, :, 0])
                        nc.sync.dma_start(
                            out=out_d[:].rearrange("(b p) -> p b", p=128), in_=dbg)

                    for blk in range(BLKS if DO_ROUTING else 0):
                        lg = logits[:, blk, :]
                        m1 = rt.tile([128, 1], f32, tag="m1")
                        nc.vector.reduce_max(m1, lg, axis=AX.X)
                        nc.vector.tensor_scalar(
                            out=eq1[:, blk, :], in0=lg, scalar1=m1, scalar2=None,
                            op0=ALU.is_equal)
                        l2 = rt.tile([128, E], f32, tag="l2")
                        nc.vector.scalar_tensor_tensor(
                            out=l2, in0=eq1[:, blk, :], scalar=-1e30, in1=lg,
                            op0=ALU.mult, op1=ALU.add)
                        m2 = rt.tile([128, 1], f32, tag="m2")
                        nc.vector.reduce_max(m2, l2, axis=AX.X)
                        nc.vector.tensor_scalar(
                            out=eq2[:, blk, :], in0=lg, scalar1=m2, scalar2=None,
                            op0=ALU.is_equal)
                        nc.vector.tensor_sub(dm_all[:, blk:blk + 1], m2, m1)

                    # s2 = sigmoid(m2 - m1), s1 = 1 - s2  (softmax over top-2)
                    if DO_ROUTING:
                        nc.scalar.activation(out=s2_all, in_=dm_all, func=ACT.Sigmoid)
                        nc.vector.tensor_scalar(
                            out=s1_all, in0=s2_all, scalar1=-1.0, scalar2=1.0,
                            op0=ALU.mult, op1=ALU.add)
                    for blk in range(BLKS if DO_ROUTING else 0):
                        t1 = rt.tile([128, E], f32, tag="t1")
                        nc.vector.tensor_scalar(
                            out=t1, in0=eq1[:, blk, :], scalar1=s1_all[:, blk:blk + 1],
                            scalar2=None, op0=ALU.mult)
                        nc.vector.tensor_scalar(
                            out=w_tok[:, blk, :], in0=eq2[:, blk, :],
                            scalar1=s2_all[:, blk:blk + 1], scalar2=None, op0=ALU.mult)
                        nc.vector.tensor_add(w_tok[:, blk, :], w_tok[:, blk, :], t1)

                if KCUT == 2:
                    dbg = singles.tile([128, BLKS], f32)
                    nc.vector.tensor_copy(out=dbg, in_=w_tok[:, :, 0])
                    nc.sync.dma_start(
                        out=out_d[:].rearrange("(b p) -> p b", p=128), in_=dbg)

                # main loop: per expert, stream w1 + build w2sum, 8 token blocks
                with tc.tile_pool(name="w1p", bufs=2) as w1p, \
                     tc.tile_pool(name="w2p", bufs=2) as w2p, \
                     tc.tile_pool(name="w2r", bufs=2) as w2rp, \
                     tc.tile_pool(name="w2b", bufs=2) as w2bp, \
                     tc.tile_pool(name="gp", bufs=2) as gp, \
                     tc.tile_pool(name="psm", bufs=2, space="PSUM") as psm:
                    for e in range(E if DO_MAIN else 0):
                        w1t = w1p.tile([128, KC, H], dt.float16, tag="w1")
                        nc.sync.dma_start(
                            out=w1t, in_=w1_d[e].rearrange("(k p) h -> p k h", p=128))

                        # w2sum[e]: reduce w2[e] over d in 4 chunks of 4 h-groups
                        w2r = w2rp.tile([128, 16], f32, tag="w2r")
                        for q in range(4):
                            w2t = w2p.tile([128, 4, D], f32, tag="w2")
                            nc.sync.dma_start(
                                out=w2t,
                                in_=w2_d[e, q * 512:(q + 1) * 512, :].rearrange(
                                    "(c p) d -> p c d", p=128))
                            nc.vector.reduce_sum(w2r[:, q * 4:(q + 1) * 4], w2t, axis=AX.X)
                        nc.sync.dma_start(
                            out=w2s_d[e].rearrange("(c p) -> p c", p=128), in_=w2r)
                        w2e = w2s_d[e]
                        w2sum_bc = w2bp.tile([128, H], f32, tag="w2b")
                        nc.gpsimd.dma_start(
                            out=w2sum_bc,
                            in_=bass.AP(tensor=w2e.tensor, offset=w2e.offset,
                                        ap=[[0, 128]] + [list(a) for a in w2e.ap]),
                        )

                        for blk in range(BLKS):
                            hp = psm.tile([128, H], f32, tag="hp")
                            for k in range(KC):
                                lhsT = xT[:, k, blk * 128:(blk + 1) * 128]
                                for n in range(NH):
                                    nc.tensor.matmul(
                                        hp[:, n * 512:(n + 1) * 512], lhsT,
                                        w1t[:, k, n * 512:(n + 1) * 512],
                                        start=(k == 0),
                                        stop=(k == KC - 1 and not has_b1))
                            if has_b1:
                                for n in range(NH):
                                    nc.tensor.matmul(
                                        hp[:, n * 512:(n + 1) * 512], ones_row,
                                        b1_sb[:, e, n * 512:(n + 1) * 512],
                                        start=False, stop=True)
                            g_sb = gp.tile([128, H], f32, tag="g")
                            nc.scalar.activation(out=g_sb, in_=hp, func=ACT.Gelu)
                            # phat[t, blk, e] = sum_h g * w2sum[e, h] (fused)
                            nc.vector.scalar_tensor_tensor(
                                out=g_sb, in0=g_sb, scalar=1.0, in1=w2sum_bc,
                                op0=ALU.mult, op1=ALU.mult,
                                accum_out=phat[:, blk, e:e + 1])

                if KCUT == 3:
                    dbg = singles.tile([128, BLKS], f32)
                    nc.vector.tensor_copy(out=dbg, in_=phat[:, :, 0])
                    nc.sync.dma_start(
                        out=out_d[:].rearrange("(b p) -> p b", p=128), in_=dbg)

                # y[t] = sum_e w_tok[t, e] * phat[t, e]
                with tc.tile_pool(name="fin", bufs=2) as fin, \
                     tc.tile_pool(name="psf", bufs=1, space="PSUM") as psf:
                    for blk in range(BLKS if KCUT == 0 else 0):
                        nc.vector.tensor_add(
                            phat[:, blk, :], phat[:, blk, :], b2s_bc)
                        sc = fin.tile([128, E], f32, tag="sc")
                        nc.vector.tensor_mul(sc, phat[:, blk, :], w_tok[:, blk, :])
                        nc.vector.reduce_sum(y_sb[:, blk:blk + 1], sc, axis=AX.X)

                    # log_softmax over all 1024 values of this batch row
                    if KCUT == 0:
                        _tail(nc, tc, psf, fin, singles, out_d, ident, ones_col,
                              ones_row, y_sb, f32, ALU, ACT, AX)

    nc.compile()
    return nc


def get_nc(has_b1: bool):
    key = (has_b1, KCUT)
    if key not in _CACHE:
        _CACHE[key] = _build(has_b1)
    return _CACHE[key]


def make_in_maps(x, gate_w, gate_b, w1, b1, w2, b2):
    f = np.float32
    common = {
        "ident128": np.eye(128, dtype=f),
        "gate_w_t": np.ascontiguousarray(np.asarray(gate_w, f).T),
        "gate_b": np.ascontiguousarray(gate_b, f),
        "w1": np.ascontiguousarray(np.asarray(w1, f)).astype(np.float16),
        "b1": np.ascontiguousarray(b1, f),
        "w2": np.ascontiguousarray(w2, f),
        "b2": np.ascontiguousarray(b2, f),
    }
    return [
        {"x_shard": np.ascontiguousarray(x[c], f), **common}
        for c in range(B)
    ]


def kernel(x, gate_w, gate_b, w1, b1, w2, b2):
    from concourse.bass_utils import run_bass_kernel_spmd

    x = np.asarray(x)
    has_b1 = bool(np.any(np.asarray(b1)))
    nc = get_nc(has_b1)
    in_maps = make_in_maps(x, gate_w, gate_b, w1, b1, w2, b2)
    res = run_bass_kernel_spmd(nc, in_maps, core_ids=list(range(B)))
    return np.stack([res.results[c]["out"] for c in range(B)]).astype(np.float32)


import concourse.bass as bass  # noqa: E402  (used by _build at call time)



# revision 13
# speedup vs baseline: 1.1186x; 1.1186x over previous
"""MoE transformer MLP (top-2 of 8 experts) + log_softmax head, on 8 trn2 cores.

Expert-parallel sparse dispatch. Core c owns batch row c for routing and
expert c for compute:
  1. Gate on PE in fp16 hi/lo pairs (exact to ~fp32) from host-transposed
     xT fp16 + residual; top-2 + softmax weights on DVE.
  2. Per 128-token chunk, ranks within (chunk, expert) via a strictly-lower-
     triangular matmul; a one-hot "selection" matrix (token x slot) built on
     DVE scatters (gid+1, w_tok) pairs into a per-expert padded slot table
     via two tiny PE matmuls per chunk (capacity 48/chunk/expert).
  3. AllToAll exchanges slot tables; each core dma_gathers its expert's
     routed tokens (3072 slots incl padding) straight into transposed
     matmul layout from a full fp16 copy of x.
  4. Sparse GEMM (24 jtiles x [128tok x 512d x 2048h]) in fp16, gelu on ACT,
     h . w2sum on DVE (w2 is algebraically folded: only sum_d w2 is needed
     because the model output is log_softmax_S(sum_d y)).
  5. Weighted contribs AllToAll back; home core combines via the same
     one-hot matrices (no scatter), then log_softmax over its 1024 tokens.
"""

import sys

for _p in ("/opt/trn_rl_repo",):
    if _p not in sys.path:
        sys.path.insert(0, _p)

import numpy as np

B, S, D, H, E = 8, 1024, 512, 2048, 8
TLOC = S
NCH = TLOC // 128   # 8 token chunks per core
KC = D // 128       # 4 contraction chunks
NH = H // 512       # 4 psum-bank-wide slices of H
CAPC = 48           # capacity per (core-row, expert, 128-chunk)
SEG = NCH * CAPC    # 384: slots per (row, expert)
J = E * SEG         # 3072: slots per expert globally
NJ = J // 128       # 24 jtiles
NGS = 4             # dma_gather split
GI = J // NGS       # idxs per gather call

_CACHE = {}


def _tail(nc, tc, psf, fin, out_d, ident, ones_col, ones_row, y_sb, f32, ALU, ACT, AX):
    # log_softmax over the 1024 values in y_sb ([128, 8], col-major chunks)
    yT_ps = psf.tile([NCH, 128], f32, tag="yT")
    nc.tensor.transpose(yT_ps, y_sb, ident)
    yT_sb = fin.tile([NCH, 128], f32, tag="yTs")
    nc.vector.tensor_copy(out=yT_sb, in_=yT_ps)
    bmax = fin.tile([NCH, 1], f32, tag="bmax")
    nc.vector.reduce_max(bmax, yT_sb, axis=AX.X)
    bT_ps = psf.tile([1, NCH], f32, tag="bT")
    nc.tensor.transpose(bT_ps, bmax, ident[:NCH, :NCH])
    brow = fin.tile([1, NCH], f32, tag="brow")
    nc.vector.tensor_copy(out=brow, in_=bT_ps)
    gmax = fin.tile([1, 1], f32, tag="gmax")
    nc.vector.reduce_max(gmax, brow, axis=AX.X)
    gmax_ps = psf.tile([128, 1], f32, tag="gmaxp")
    nc.tensor.matmul(gmax_ps, ones_row, gmax, start=True, stop=True)
    gmax_bc = fin.tile([128, 1], f32, tag="gmaxb")
    nc.vector.tensor_copy(out=gmax_bc, in_=gmax_ps)
    esb = fin.tile([128, NCH], f32, tag="esb")
    nc.vector.tensor_scalar(
        out=esb, in0=y_sb, scalar1=gmax_bc, scalar2=None, op0=ALU.subtract)
    ex = fin.tile([128, NCH], f32, tag="ex")
    rowsum = fin.tile([128, 1], f32, tag="rowsum")
    nc.scalar.activation(out=ex, in_=esb, func=ACT.Exp, accum_out=rowsum)
    tot = psf.tile([1, 1], f32, tag="tot")
    nc.tensor.matmul(tot, ones_col, rowsum, start=True, stop=True)
    lse = fin.tile([1, 1], f32, tag="lse")
    nc.scalar.activation(out=lse, in_=tot, func=ACT.Ln)
    nc.vector.tensor_add(lse, lse, gmax)
    lse_ps = psf.tile([128, 1], f32, tag="lsep")
    nc.tensor.matmul(lse_ps, ones_row, lse, start=True, stop=True)
    lse_bc = fin.tile([128, 1], f32, tag="lseb")
    nc.vector.tensor_copy(out=lse_bc, in_=lse_ps)
    outsb = fin.tile([128, NCH], f32, tag="outsb")
    nc.vector.tensor_scalar(
        out=outsb, in0=y_sb, scalar1=lse_bc, scalar2=None, op0=ALU.subtract)
    nc.sync.dma_start(
        out=out_d[:].rearrange("(b p) -> p b", p=128), in_=outsb)


def _build(has_b1: bool):
    import concourse.bass as bass
    import concourse.tile as tile
    import concourse.mybir as mybir
    from concourse import bacc

    dt = mybir.dt
    f32 = dt.float32
    f16 = dt.float16
    i16 = dt.int16
    ALU = mybir.AluOpType
    ACT = mybir.ActivationFunctionType
    AX = mybir.AxisListType
    RG = [list(range(B))]

    nc = bacc.Bacc(None, target_bir_lowering=False)

    with tile.TileContext(nc) as tc:
        with tc.tile_pool(name="dram", bufs=1, space="DRAM") as dram:
            xT_d = dram.tile([D, TLOC], f16, kind="ExternalInput", name="xT16", uniquify=False)
            xTr_d = dram.tile([D, TLOC], f16, kind="ExternalInput", name="xTr16", uniquify=False)
            xall_d = dram.tile([B * S, D], f16, kind="ExternalInput", name="x_all16", uniquify=False)
            gwcat_d = dram.tile([D, 16], f16, kind="ExternalInput", name="gwcat16", uniquify=False)
            gb_d = dram.tile([E], f32, kind="ExternalInput", name="gate_b", uniquify=False)
            w1_d = dram.tile([D, H], f16, kind="ExternalInput", name="w1c", uniquify=False)
            b1_d = dram.tile([H], f32, kind="ExternalInput", name="b1c", uniquify=False)
            w2_d = dram.tile([H, D], f32, kind="ExternalInput", name="w2c", uniquify=False)
            b2_d = dram.tile([D], f32, kind="ExternalInput", name="b2c", uniquify=False)
            id_d = dram.tile([128, 128], f32, kind="ExternalInput", name="ident128", uniquify=False)
            lt_d = dram.tile([128, 128], f32, kind="ExternalInput", name="lstrict", uniquify=False)
            on_d = dram.tile([128, 128], f32, kind="ExternalInput", name="ones128", uniquify=False)
            io48_d = dram.tile([128, CAPC], f32, kind="ExternalInput", name="iota48", uniquify=False)
            gid_d = dram.tile([128, NCH], f32, kind="ExternalInput", name="gidmat", uniquify=False)
            out_d = dram.tile([TLOC], f32, kind="ExternalOutput", name="out", uniquify=False)

            payload_d = dram.tile([B * 2 * SEG], f32, name="payload")
            recv_d = dram.tile([B * 2 * SEG], f32, name="recv")
            wtflat_d = dram.tile([J], f32, name="wtflat")
            idflat_d = dram.tile([J], f32, name="idflat")
            cont_d = dram.tile([J], f32, name="cont")
            recv2_d = dram.tile([J], f32, name="recv2")
            w2s_d = dram.tile([H], f32, name="w2s_scratch")
            b2s_d = dram.tile([1], f32, name="b2s_scratch")

            with tc.tile_pool(name="singles", bufs=1) as singles:
                ident = singles.tile([128, 128], f32)
                nc.sync.dma_start(out=ident, in_=id_d[:])
                lstrict = singles.tile([128, 128], f32)
                nc.sync.dma_start(out=lstrict, in_=lt_d[:])
                ones128 = singles.tile([128, 128], f32)
                nc.sync.dma_start(out=ones128, in_=on_d[:])
                iota48 = singles.tile([128, CAPC], f32)
                nc.sync.dma_start(out=iota48, in_=io48_d[:])
                gidmat = singles.tile([128, NCH], f32)
                nc.sync.dma_start(out=gidmat, in_=gid_d[:])
                ones_col = singles.tile([128, 1], f32)
                nc.vector.memset(ones_col, 1.0)
                ones_row = singles.tile([1, 128], f32)
                nc.vector.memset(ones_row, 1.0)
                gb_sb = singles.tile([E, 1], f32)
                nc.sync.dma_start(out=gb_sb, in_=gb_d[:, None])

                # ------- expert-side weights (independent of routing) -------
                w1_sb = singles.tile([128, KC, H], f16)
                nc.sync.dma_start(
                    out=w1_sb, in_=w1_d[:].rearrange("(k p) h -> p k h", p=128))
                if has_b1:
                    b1_sb = singles.tile([1, H], f32)
                    nc.sync.dma_start(out=b1_sb, in_=b1_d[None, :])

                # w2sum[h] = sum_d w2[h, d]; then broadcast [128, H]
                w2s_bc = singles.tile([128, H], f32)
                with tc.tile_pool(name="w2p", bufs=2) as w2p:
                    w2r = singles.tile([128, 16], f32)
                    for q in range(4):
                        w2t = w2p.tile([128, 4, D], f32, tag="w2")
                        nc.sync.dma_start(
                            out=w2t,
                            in_=w2_d[q * 512:(q + 1) * 512, :].rearrange(
                                "(c p) d -> p c d", p=128))
                        nc.vector.reduce_sum(w2r[:, q * 4:(q + 1) * 4], w2t, axis=AX.X)
                    nc.sync.dma_start(
                        out=w2s_d[:].rearrange("(c p) -> p c", p=128), in_=w2r)
                    nc.gpsimd.dma_start(
                        out=w2s_bc,
                        in_=bass.AP(tensor=w2s_d.tensor, offset=w2s_d.offset,
                                    ap=[[0, 128]] + [list(a) for a in w2s_d.ap]))

                # b2sum = sum_d b2[d], broadcast [128, 1]
                b2s_bc = singles.tile([128, 1], f32)
                with tc.tile_pool(name="b2p", bufs=1) as b2p, \
                     tc.tile_pool(name="b2ps", bufs=1, space="PSUM") as b2ps:
                    b2t = b2p.tile([128, 4], f32, tag="b2")
                    nc.sync.dma_start(
                        out=b2t, in_=b2_d[:].rearrange("(c p) -> p c", p=128))
                    b2part = b2p.tile([128, 1], f32, tag="b2s")
                    nc.vector.reduce_sum(b2part, b2t, axis=AX.X)
                    tot_ps = b2ps.tile([1, 1], f32, tag="tot")
                    nc.tensor.matmul(tot_ps, b2part, ones_col, start=True, stop=True)
                    tot_sb = b2p.tile([1, 1], f32, tag="tots")
                    nc.vector.tensor_copy(out=tot_sb, in_=tot_ps)
                    bc_ps = b2ps.tile([128, 1], f32, tag="bc")
                    nc.tensor.matmul(bc_ps, ones_row, tot_sb, start=True, stop=True)
                    nc.vector.tensor_copy(out=b2s_bc, in_=bc_ps)

                # ---------------- gate: logitsT = gw.T @ x (hi/lo fp16) ----
                xTh = singles.tile([128, KC, TLOC], f16)
                nc.sync.dma_start(
                    out=xTh, in_=xT_d[:].rearrange("(k p) t -> p k t", p=128))
                xTr = singles.tile([128, KC, TLOC], f16)
                nc.sync.dma_start(
                    out=xTr, in_=xTr_d[:].rearrange("(k p) t -> p k t", p=128))
                gwcat = singles.tile([128, KC, 16], f16)
                nc.sync.dma_start(
                    out=gwcat, in_=gwcat_d[:].rearrange("(k p) e -> p k e", p=128))

                logitsT = singles.tile([E, TLOC], f32)
                logits = singles.tile([128, NCH, E], f32)
                with tc.tile_pool(name="gps", bufs=1, space="PSUM") as gps, \
                     tc.tile_pool(name="gsb", bufs=2) as gsb:
                    lps_a = gps.tile([E, TLOC], f32, tag="lpsa")
                    lps_c = gps.tile([E, TLOC], f32, tag="lpsc")
                    lps_b = gps.tile([E, TLOC], f32, tag="lpsb")
                    for ps, wsl, xx in ((lps_a, slice(0, 8), xTh),
                                        (lps_c, slice(8, 16), xTh),
                                        (lps_b, slice(0, 8), xTr)):
                        for tt in range(2):
                            sl = slice(tt * 512, (tt + 1) * 512)
                            for k in range(KC):
                                nc.tensor.matmul(
                                    ps[:, sl], gwcat[:, k, wsl], xx[:, k, sl],
                                    start=(k == 0), stop=(k == KC - 1))
                    t1 = gsb.tile([E, TLOC], f32, tag="t1")
                    nc.vector.tensor_copy(out=t1, in_=lps_a)
                    nc.vector.tensor_add(t1, t1, lps_c)
                    nc.vector.tensor_add(t1, t1, lps_b)
                    nc.vector.tensor_scalar(
                        out=logitsT, in0=t1, scalar1=gb_sb, scalar2=None,
                        op0=ALU.add)

                # transpose logitsT -> logits [128, chunk, e]
                with tc.tile_pool(name="tps", bufs=2, space="PSUM") as tps:
                    for k in range(NCH):
                        tp = tps.tile([128, E], f32, tag="tp")
                        nc.tensor.transpose(
                            tp, logitsT[:, k * 128:(k + 1) * 128], ident[:E, :E])
                        nc.vector.tensor_copy(out=logits[:, k, :], in_=tp)

                # ---------------- routing (top-2 + softmax weights) --------
                eq1 = singles.tile([128, NCH, E], f32)
                eq2 = singles.tile([128, NCH, E], f32)
                dm_all = singles.tile([128, NCH], f32)
                s1_all = singles.tile([128, NCH], f32)
                s2_all = singles.tile([128, NCH], f32)
                with tc.tile_pool(name="rt", bufs=4) as rt:
                    for k in range(NCH):
                        lg = logits[:, k, :]
                        m1 = rt.tile([128, 1], f32, tag="m1")
                        nc.vector.reduce_max(m1, lg, axis=AX.X)
                        nc.vector.tensor_scalar(
                            out=eq1[:, k, :], in0=lg, scalar1=m1, scalar2=None,
                            op0=ALU.is_equal)
                        l2 = rt.tile([128, E], f32, tag="l2")
                        nc.vector.scalar_tensor_tensor(
                            out=l2, in0=eq1[:, k, :], scalar=-1e30, in1=lg,
                            op0=ALU.mult, op1=ALU.add)
                        m2 = rt.tile([128, 1], f32, tag="m2")
                        nc.vector.reduce_max(m2, l2, axis=AX.X)
                        nc.vector.tensor_scalar(
                            out=eq2[:, k, :], in0=lg, scalar1=m2, scalar2=None,
                            op0=ALU.is_equal)
                        nc.vector.tensor_sub(dm_all[:, k:k + 1], m2, m1)
                    nc.scalar.activation(out=s2_all, in_=dm_all, func=ACT.Sigmoid)
                    nc.vector.tensor_scalar(
                        out=s1_all, in0=s2_all, scalar1=-1.0, scalar2=1.0,
                        op0=ALU.mult, op1=ALU.add)

                # ---------------- ranks within (chunk, expert) -------------
                mask = singles.tile([128, NCH, E], f32)
                nc.vector.tensor_add(
                    mask.rearrange("p a b -> p (a b)"),
                    eq1.rearrange("p a b -> p (a b)"),
                    eq2.rearrange("p a b -> p (a b)"))
                rank = singles.tile([128, NCH, E], f32)
                with tc.tile_pool(name="rps", bufs=1, space="PSUM") as rps:
                    rk_ps = rps.tile([128, NCH * E], f32, tag="rk")
                    nc.tensor.matmul(
                        rk_ps, lstrict, mask.rearrange("p a b -> p (a b)"),
                        start=True, stop=True)
                    nc.vector.tensor_copy(
                        out=rank.rearrange("p a b -> p (a b)"), in_=rk_ps)

                # rankm = rank*eq + eq - 1  (=-1 when not routed)
                psel1 = singles.tile([128, NCH, E, CAPC], f32)
                psel2 = singles.tile([128, NCH, E, CAPC], f32)
                rkp1 = singles.tile([128, NCH, E], f32)
                nc.vector.tensor_scalar(
                    out=rkp1.rearrange("p a b -> p (a b)"),
                    in0=rank.rearrange("p a b -> p (a b)"),
                    scalar1=1.0, scalar2=None, op0=ALU.add)
                for sel, eqm, pselm in ((0, eq1, psel1), (1, eq2, psel2)):
                    rm = singles.tile([128, NCH, E], f32)
                    nc.vector.tensor_mul(
                        rm.rearrange("p a b -> p (a b)"),
                        eqm.rearrange("p a b -> p (a b)"),
                        rkp1.rearrange("p a b -> p (a b)"))
                    nc.vector.tensor_scalar(
                        out=rm.rearrange("p a b -> p (a b)"),
                        in0=rm.rearrange("p a b -> p (a b)"),
                        scalar1=-1.0, scalar2=None, op0=ALU.add)
                    for k in range(NCH):
                        for e in range(E):
                            nc.vector.tensor_scalar(
                                out=pselm[:, k, e, :], in0=iota48,
                                scalar1=rm[:, k, e:e + 1], scalar2=None,
                                op0=ALU.is_equal)

                # ---------------- dispatch scatter via PE ------------------
                pairs1 = singles.tile([128, NCH, 2], f32)
                pairs2 = singles.tile([128, NCH, 2], f32)
                nc.vector.tensor_copy(out=pairs1[:, :, 0], in_=gidmat)
                nc.vector.tensor_copy(out=pairs1[:, :, 1], in_=s1_all)
                nc.vector.tensor_copy(out=pairs2[:, :, 0], in_=gidmat)
                nc.vector.tensor_copy(out=pairs2[:, :, 1], in_=s2_all)

                disp_sb = singles.tile([2, NCH, E * CAPC], f32)
                with tc.tile_pool(name="dps", bufs=1, space="PSUM") as dps:
                    disp_ps = dps.tile([2, NCH, 512], f32, tag="disp")
                    for k in range(NCH):
                        nc.tensor.matmul(
                            disp_ps[:, k, :E * CAPC], pairs1[:, k, :],
                            psel1[:, k].rearrange("p e r -> p (e r)"),
                            start=True, stop=False)
                        nc.tensor.matmul(
                            disp_ps[:, k, :E * CAPC], pairs2[:, k, :],
                            psel2[:, k].rearrange("p e r -> p (e r)"),
                            start=False, stop=True)
                    for k in range(NCH):
                        nc.vector.tensor_copy(
                            out=disp_sb[:, k, :], in_=disp_ps[:, k, :E * CAPC])

                # payload[e, s, k, r] = disp_sb[s, k, (e, r)]
                for e in range(E):
                    nc.sync.dma_start(
                        out=payload_d[e * 2 * SEG:(e + 1) * 2 * SEG].rearrange(
                            "(s k r) -> s k r", s=2, k=NCH, r=CAPC),
                        in_=disp_sb[:, :, e * CAPC:(e + 1) * CAPC])

                nc.gpsimd.collective_compute(
                    "AllToAll", mybir.AluOpType.bypass, replica_groups=RG,
                    ins=[payload_d[:].opt()], outs=[recv_d[:].opt()])

                # ---------------- extract idx + wt from recv ---------------
                # recv[b, s, k, r]; gather slot j = b*SEG + k*CAPC + r
                rv = recv_d[:].rearrange("(b s q) -> b s q", b=B, s=2, q=SEG)
                nc.sync.dma_start(out=idflat_d[:], in_=rv[:, 0, :])
                nc.sync.dma_start(out=wtflat_d[:], in_=rv[:, 1, :])

                idxf = singles.tile([128, J // 16], f32)
                idwrap = idflat_d[:].rearrange("(f p) -> p f", p=16)
                for g in range(8):
                    nc.sync.dma_start(out=idxf[16 * g:16 * (g + 1)], in_=idwrap)
                idx16 = singles.tile([128, J // 16], i16)
                nc.vector.tensor_scalar(
                    out=idxf, in0=idxf,
                    scalar1=-1.0, scalar2=0.0, op0=ALU.add, op1=ALU.max)
                nc.vector.tensor_copy(out=idx16, in_=idxf)

                wt_sb = singles.tile([128, NJ], f32)
                nc.sync.dma_start(
                    out=wt_sb, in_=wtflat_d[:].rearrange("(c p) -> p c", p=128))

                # ---------------- gather routed tokens ---------------------
                xTg_t = [singles.tile([128, KC, GI], f16, name=f"xTg{g}")
                         for g in range(NGS)]
                for g in range(NGS):
                    nc.gpsimd.dma_gather(
                        xTg_t[g][:], xall_d[:],
                        idx16[:, g * (GI // 16):(g + 1) * (GI // 16)],
                        GI, GI, D, transpose=True)

                # ---------------- main sparse GEMM -------------------------
                phat = singles.tile([128, NJ], f32)
                with tc.tile_pool(name="gp", bufs=2) as gp, \
                     tc.tile_pool(name="psm", bufs=2, space="PSUM") as psm:
                    JT_PER_G = GI // 128
                    for jt in range(NJ):
                        hp = psm.tile([128, H], f32, tag="hp")
                        xTg = xTg_t[jt // JT_PER_G]
                        jl = jt % JT_PER_G
                        for k in range(KC):
                            lhsT = xTg[:, k, jl * 128:(jl + 1) * 128]
                            for n in range(NH):
                                nc.tensor.matmul(
                                    hp[:, n * 512:(n + 1) * 512], lhsT,
                                    w1_sb[:, k, n * 512:(n + 1) * 512],
                                    start=(k == 0),
                                    stop=(k == KC - 1 and not has_b1))
                        if has_b1:
                            for n in range(NH):
                                nc.tensor.matmul(
                                    hp[:, n * 512:(n + 1) * 512], ones_row,
                                    b1_sb[:, n * 512:(n + 1) * 512],
                                    start=False, stop=True)
                        g_sb = gp.tile([128, H], f32, tag="g")
                        nc.scalar.activation(out=g_sb, in_=hp, func=ACT.Gelu)
                        nc.vector.scalar_tensor_tensor(
                            out=g_sb, in0=g_sb, scalar=1.0, in1=w2s_bc,
                            op0=ALU.mult, op1=ALU.mult,
                            accum_out=phat[:, jt:jt + 1])

                # contribs = (phat + b2sum) * wt, back in slot order
                cont_sb = singles.tile([128, NJ], f32)
                nc.vector.tensor_scalar(
                    out=cont_sb, in0=phat, scalar1=b2s_bc, scalar2=None,
                    op0=ALU.add)
                nc.vector.tensor_mul(cont_sb, cont_sb, wt_sb)
                nc.sync.dma_start(
                    out=cont_d[:].rearrange("(c p) -> p c", p=128), in_=cont_sb)

                nc.gpsimd.collective_compute(
                    "AllToAll", mybir.AluOpType.bypass, replica_groups=RG,
                    ins=[cont_d[:].opt()], outs=[recv2_d[:].opt()])

                # ---------------- combine at home core ---------------------
                # recv2 flat slot (e, k, r); for chunk k pick [e, r] block
                y_sb = singles.tile([128, NCH], f32)
                with tc.tile_pool(name="cmb", bufs=4) as cmb, \
                     tc.tile_pool(name="fin", bufs=2) as fin, \
                     tc.tile_pool(name="psf", bufs=1, space="PSUM") as psf:
                    for k in range(NCH):
                        rrow = cmb.tile([1, E, CAPC], f32, tag="rrow")
                        nc.sync.dma_start(
                            out=rrow,
                            in_=recv2_d[:].rearrange(
                                "(e k r) -> e k r", e=E, k=NCH)[None, :, k, :])
                        rbc = cmb.tile([128, E * CAPC], f32, tag="rbc")
                        nc.gpsimd.partition_broadcast(
                            rbc, rrow.rearrange("p a b -> p (a b)"))
                        y1 = cmb.tile([128, 1], f32, tag="y1")
                        y2 = cmb.tile([128, 1], f32, tag="y2")
                        sc1 = cmb.tile([128, E * CAPC], f32, tag="sc1")
                        nc.vector.scalar_tensor_tensor(
                            out=sc1, in0=psel1[:, k].rearrange("p e r -> p (e r)"),
                            scalar=1.0, in1=rbc, op0=ALU.mult, op1=ALU.mult,
                            accum_out=y1)
                        sc2 = cmb.tile([128, E * CAPC], f32, tag="sc2")
                        nc.vector.scalar_tensor_tensor(
                            out=sc2, in0=psel2[:, k].rearrange("p e r -> p (e r)"),
                            scalar=1.0, in1=rbc, op0=ALU.mult, op1=ALU.mult,
                            accum_out=y2)
                        nc.vector.tensor_add(y_sb[:, k:k + 1], y1, y2)

                    _tail(nc, tc, psf, fin, out_d, ident, ones_col, ones_row,
                          y_sb, f32, ALU, ACT, AX)

    nc.compile()
    return nc


def get_nc(has_b1: bool):
    key = (has_b1,)
    if key not in _CACHE:
        _CACHE[key] = _build(has_b1)
    return _CACHE[key]


def make_in_maps(x, gate_w, gate_b, w1, b1, w2, b2):
    f = np.float32
    x = np.asarray(x, f)
    gate_w = np.asarray(gate_w, f)
    xt = x.reshape(B * S, D)
    x16 = xt.astype(np.float16)
    gw16 = gate_w.astype(np.float16)
    gwr16 = (gate_w - gw16.astype(f)).astype(np.float16)
    gwcat = np.concatenate([gw16, gwr16], axis=1)  # [D, 16]

    iota48 = np.broadcast_to(np.arange(CAPC, dtype=f), (128, CAPC)).copy()
    lstrict = np.tril(np.ones((128, 128), f), -1).T.copy()  # [p, m]: p < m
    common = {
        "x_all16": np.ascontiguousarray(x16),
        "gwcat16": np.ascontiguousarray(gwcat),
        "gate_b": np.ascontiguousarray(gate_b, dtype=f),
        "ident128": np.eye(128, dtype=f),
        "lstrict": np.ascontiguousarray(lstrict),
        "ones128": np.ones((128, 128), f),
        "iota48": iota48,
    }
    maps = []
    for c in range(B):
        xc = x[c]                      # [S, D]
        xc16 = xc.astype(np.float16)
        xr16 = (xc - xc16.astype(f)).astype(np.float16)
        gid = (np.arange(128, dtype=f)[:, None]
               + (np.arange(NCH, dtype=f) * 128)[None, :]
               + (c * S + 1))
        maps.append({
            "xT16": np.ascontiguousarray(xc16.T),
            "xTr16": np.ascontiguousarray(xr16.T),
            "w1c": np.ascontiguousarray(np.asarray(w1[c], f)).astype(np.float16),
            "b1c": np.ascontiguousarray(b1[c], dtype=f),
            "w2c": np.ascontiguousarray(w2[c], dtype=f),
            "b2c": np.ascontiguousarray(b2[c], dtype=f),
            "gidmat": np.ascontiguousarray(gid),
            **common,
        })
    return maps


def kernel(x, gate_w, gate_b, w1, b1, w2, b2):
    from concourse.bass_utils import run_bass_kernel_spmd

    has_b1 = bool(np.any(np.asarray(b1)))
    nc = get_nc(has_b1)
    in_maps = make_in_maps(x, gate_w, gate_b, w1, b1, w2, b2)
    res = run_bass_kernel_spmd(nc, in_maps, core_ids=list(range(B)))
    return np.stack([res.results[c]["out"] for c in range(B)]).astype(np.float32)


import concourse.bass as bass  # noqa: E402  (used by _build at call time)


# revision 16
# speedup vs baseline: 1.4314x; 1.2796x over previous
"""MoE transformer MLP (top-2 of 8 experts) + log_softmax head, on 8 trn2 cores.

Data-parallel sparse compute. Core c owns batch row c (1024 tokens) end to
end; the only cross-core traffic is a 64KB AllGather of per-expert w2 column
sums (each core reduces w2[c] locally), which launches early and is consumed
~50us later, so it is fully hidden under local compute.

Per core:
  1. Gate on PE in fp16 hi/lo pairs (exact to ~fp32) from host-transposed
     xT fp16 + residual; top-2 + softmax weights on DVE.
  2. Per 128-token chunk, ranks within (chunk, expert) via a strictly-lower-
     triangular matmul; one-hot token-by-slot matrices on DVE; two tiny PE
     matmuls per chunk scatter (token_id+1, w_tok) into a padded slot table
     (capacity 48 per chunk*expert; 3072 slots = 24 jtiles total, vs 8192
     dense jtile-equivalents).
  3. dma_gather pulls the routed tokens straight into transposed matmul
     layout from an fp16 copy of this core's x.
  4. Sparse GEMM in fp16 (only routed (token, expert) pairs), gelu on ACT,
     then h . w2sum on DVE; w2 is algebraically folded: the model output is
     log_softmax_S(sum_d y) and sum_d(h @ w2[e] + b2[e]) = h . w2sum[e] +
     b2sum[e], so the [T, D] expert outputs are never materialized.
  5. Combine with the same one-hot matrices (no scatter), log_softmax tail.
"""

import sys

for _p in ("/opt/trn_rl_repo",):
    if _p not in sys.path:
        sys.path.insert(0, _p)

import numpy as np

B, S, D, H, E = 8, 1024, 512, 2048, 8
TLOC = S
NCH = TLOC // 128   # 8 token chunks per core
KC = D // 128       # 4 contraction chunks
NH = H // 512       # 4 psum-bank-wide slices of H
CAPC = 48           # capacity per (expert, 128-chunk)
SEG = NCH * CAPC    # 384 slots per expert
J = E * SEG         # 3072 slots total
NJ = J // 128       # 24 jtiles
JPE = SEG // 128    # 3 jtiles per expert
NGS = 4             # dma_gather split
GI = J // NGS

_CACHE = {}


def _tail(nc, tc, psf, fin, out_d, ident, ones_col, ones_row, y_sb, f32, ALU, ACT, AX):
    # log_softmax over the 1024 values in y_sb ([128, 8], col-major chunks)
    yT_ps = psf.tile([NCH, 128], f32, tag="yT")
    nc.tensor.transpose(yT_ps, y_sb, ident)
    yT_sb = fin.tile([NCH, 128], f32, tag="yTs")
    nc.vector.tensor_copy(out=yT_sb, in_=yT_ps)
    bmax = fin.tile([NCH, 1], f32, tag="bmax")
    nc.vector.reduce_max(bmax, yT_sb, axis=AX.X)
    bT_ps = psf.tile([1, NCH], f32, tag="bT")
    nc.tensor.transpose(bT_ps, bmax, ident[:NCH, :NCH])
    brow = fin.tile([1, NCH], f32, tag="brow")
    nc.vector.tensor_copy(out=brow, in_=bT_ps)
    gmax = fin.tile([1, 1], f32, tag="gmax")
    nc.vector.reduce_max(gmax, brow, axis=AX.X)
    gmax_ps = psf.tile([128, 1], f32, tag="gmaxp")
    nc.tensor.matmul(gmax_ps, ones_row, gmax, start=True, stop=True)
    gmax_bc = fin.tile([128, 1], f32, tag="gmaxb")
    nc.vector.tensor_copy(out=gmax_bc, in_=gmax_ps)
    esb = fin.tile([128, NCH], f32, tag="esb")
    nc.vector.tensor_scalar(
        out=esb, in0=y_sb, scalar1=gmax_bc, scalar2=None, op0=ALU.subtract)
    ex = fin.tile([128, NCH], f32, tag="ex")
    rowsum = fin.tile([128, 1], f32, tag="rowsum")
    nc.scalar.activation(out=ex, in_=esb, func=ACT.Exp, accum_out=rowsum)
    tot = psf.tile([1, 1], f32, tag="tot")
    nc.tensor.matmul(tot, ones_col, rowsum, start=True, stop=True)
    lse = fin.tile([1, 1], f32, tag="lse")
    nc.scalar.activation(out=lse, in_=tot, func=ACT.Ln)
    nc.vector.tensor_add(lse, lse, gmax)
    lse_ps = psf.tile([128, 1], f32, tag="lsep")
    nc.tensor.matmul(lse_ps, ones_row, lse, start=True, stop=True)
    lse_bc = fin.tile([128, 1], f32, tag="lseb")
    nc.vector.tensor_copy(out=lse_bc, in_=lse_ps)
    outsb = fin.tile([128, NCH], f32, tag="outsb")
    nc.vector.tensor_scalar(
        out=outsb, in0=y_sb, scalar1=lse_bc, scalar2=None, op0=ALU.subtract)
    nc.sync.dma_start(
        out=out_d[:].rearrange("(b p) -> p b", p=128), in_=outsb)


def _build(has_b1: bool):
    import concourse.bass as bass
    import concourse.tile as tile
    import concourse.mybir as mybir
    from concourse import bacc

    dt = mybir.dt
    f32 = dt.float32
    f16 = dt.float16
    i16 = dt.int16
    ALU = mybir.AluOpType
    ACT = mybir.ActivationFunctionType
    AX = mybir.AxisListType
    RG = [list(range(B))]

    nc = bacc.Bacc(None, target_bir_lowering=False)

    with tile.TileContext(nc) as tc:
        with tc.tile_pool(name="dram", bufs=1, space="DRAM") as dram:
            xT_d = dram.tile([D, TLOC], f16, kind="ExternalInput", name="xT16", uniquify=False)
            xTr_d = dram.tile([D, TLOC], f16, kind="ExternalInput", name="xTr16", uniquify=False)
            xown_d = dram.tile([TLOC, D], f16, kind="ExternalInput", name="x_own16", uniquify=False)
            gwcat_d = dram.tile([D, 16], f16, kind="ExternalInput", name="gwcat16", uniquify=False)
            gb_d = dram.tile([E], f32, kind="ExternalInput", name="gate_b", uniquify=False)
            w1_d = dram.tile([E, D, H], f16, kind="ExternalInput", name="w1all", uniquify=False)
            b1_d = dram.tile([E, H], f32, kind="ExternalInput", name="b1all", uniquify=False)
            w2_d = dram.tile([H, D], f32, kind="ExternalInput", name="w2c", uniquify=False)
            b2_d = dram.tile([E, D], f32, kind="ExternalInput", name="b2all", uniquify=False)
            id_d = dram.tile([128, 128], f32, kind="ExternalInput", name="ident128", uniquify=False)
            lt_d = dram.tile([128, 128], f32, kind="ExternalInput", name="lstrict", uniquify=False)
            io48_d = dram.tile([128, CAPC], f32, kind="ExternalInput", name="iota48", uniquify=False)
            gid_d = dram.tile([128, NCH], f32, kind="ExternalInput", name="gidmat", uniquify=False)
            out_d = dram.tile([TLOC], f32, kind="ExternalOutput", name="out", uniquify=False)

            wtflat_d = dram.tile([J], f32, name="wtflat")
            idflat_d = dram.tile([J], f32, name="idflat")
            cont_d = dram.tile([J], f32, name="cont")
            w2sown_d = dram.tile([H], f32, name="w2s_own")
            w2sall_d = dram.tile([E, H], f32, name="w2s_all")
            b2s8_d = dram.tile([E], f32, name="b2s8")

            with tc.tile_pool(name="singles", bufs=1) as singles:
                # ---- w2sum of own expert -> AllGather (launch ASAP; the
                # result is only consumed ~50us later by the combine) ----
                with tc.tile_pool(name="w2p", bufs=2) as w2p:
                    w2r = singles.tile([128, 16], f32)
                    for q in range(4):
                        w2t = w2p.tile([128, 4, D], f32, tag="w2")
                        nc.sync.dma_start(
                            out=w2t,
                            in_=w2_d[q * 512:(q + 1) * 512, :].rearrange(
                                "(c p) d -> p c d", p=128))
                        nc.vector.reduce_sum(w2r[:, q * 4:(q + 1) * 4], w2t, axis=AX.X)
                    nc.sync.dma_start(
                        out=w2sown_d[:].rearrange("(c p) -> p c", p=128), in_=w2r)
                # ---- gate inputs (needed first) ----
                xTh = singles.tile([128, KC, TLOC], f16)
                nc.sync.dma_start(
                    out=xTh, in_=xT_d[:].rearrange("(k p) t -> p k t", p=128))
                xTr = singles.tile([128, KC, TLOC], f16)
                nc.sync.dma_start(
                    out=xTr, in_=xTr_d[:].rearrange("(k p) t -> p k t", p=128))
                gwcat = singles.tile([128, KC, 16], f16)
                nc.sync.dma_start(
                    out=gwcat, in_=gwcat_d[:].rearrange("(k p) e -> p k e", p=128))
                ident = singles.tile([128, 128], f32)
                nc.sync.dma_start(out=ident, in_=id_d[:])
                lstrict = singles.tile([128, 128], f32)
                nc.sync.dma_start(out=lstrict, in_=lt_d[:])
                iota48 = singles.tile([128, CAPC], f32)
                nc.sync.dma_start(out=iota48, in_=io48_d[:])
                gidmat = singles.tile([128, NCH], f32)
                nc.sync.dma_start(out=gidmat, in_=gid_d[:])
                ones_col = singles.tile([128, 1], f32)
                nc.vector.memset(ones_col, 1.0)
                ones_row = singles.tile([1, 128], f32)
                nc.vector.memset(ones_row, 1.0)
                gb_sb = singles.tile([E, 1], f32)
                nc.sync.dma_start(out=gb_sb, in_=gb_d[:, None])

                # ---------------- gate: logitsT = gw.T @ x (hi/lo fp16) ----
                logitsT = singles.tile([E, TLOC], f32)
                logits = singles.tile([128, NCH, E], f32)
                with tc.tile_pool(name="gps", bufs=1, space="PSUM") as gps, \
                     tc.tile_pool(name="gsb", bufs=2) as gsb:
                    lps_a = gps.tile([E, TLOC], f32, tag="lpsa")
                    lps_c = gps.tile([E, TLOC], f32, tag="lpsc")
                    lps_b = gps.tile([E, TLOC], f32, tag="lpsb")
                    for ps, wsl, xx in ((lps_a, slice(0, 8), xTh),
                                        (lps_c, slice(8, 16), xTh),
                                        (lps_b, slice(0, 8), xTr)):
                        for tt in range(2):
                            sl = slice(tt * 512, (tt + 1) * 512)
                            for k in range(KC):
                                nc.tensor.matmul(
                                    ps[:, sl], gwcat[:, k, wsl], xx[:, k, sl],
                                    start=(k == 0), stop=(k == KC - 1))
                    t1 = gsb.tile([E, TLOC], f32, tag="t1")
                    nc.vector.tensor_copy(out=t1, in_=lps_a)
                    nc.vector.tensor_add(t1, t1, lps_c)
                    nc.vector.tensor_add(t1, t1, lps_b)
                    nc.vector.tensor_scalar(
                        out=logitsT, in0=t1, scalar1=gb_sb, scalar2=None,
                        op0=ALU.add)

                # transpose logitsT -> logits [128, chunk, e]
                with tc.tile_pool(name="tps", bufs=2, space="PSUM") as tps:
                    for k in range(NCH):
                        tp = tps.tile([128, E], f32, tag="tp")
                        nc.tensor.transpose(
                            tp, logitsT[:, k * 128:(k + 1) * 128], ident[:E, :E])
                        nc.vector.tensor_copy(out=logits[:, k, :], in_=tp)

                # ---------------- routing (top-2 + softmax weights) --------
                eq1 = singles.tile([128, NCH, E], f32)
                eq2 = singles.tile([128, NCH, E], f32)
                dm_all = singles.tile([128, NCH], f32)
                s1_all = singles.tile([128, NCH], f32)
                s2_all = singles.tile([128, NCH], f32)
                with tc.tile_pool(name="rt", bufs=4) as rt:
                    for k in range(NCH):
                        lg = logits[:, k, :]
                        m1 = rt.tile([128, 1], f32, tag="m1")
                        nc.vector.reduce_max(m1, lg, axis=AX.X)
                        nc.vector.tensor_scalar(
                            out=eq1[:, k, :], in0=lg, scalar1=m1, scalar2=None,
                            op0=ALU.is_equal)
                        l2 = rt.tile([128, E], f32, tag="l2")
                        nc.vector.scalar_tensor_tensor(
                            out=l2, in0=eq1[:, k, :], scalar=-1e30, in1=lg,
                            op0=ALU.mult, op1=ALU.add)
                        m2 = rt.tile([128, 1], f32, tag="m2")
                        nc.vector.reduce_max(m2, l2, axis=AX.X)
                        nc.vector.tensor_scalar(
                            out=eq2[:, k, :], in0=lg, scalar1=m2, scalar2=None,
                            op0=ALU.is_equal)
                        nc.vector.tensor_sub(dm_all[:, k:k + 1], m2, m1)
                    nc.scalar.activation(out=s2_all, in_=dm_all, func=ACT.Sigmoid)
                    nc.vector.tensor_scalar(
                        out=s1_all, in0=s2_all, scalar1=-1.0, scalar2=1.0,
                        op0=ALU.mult, op1=ALU.add)

                # ---------------- ranks within (chunk, expert) -------------
                mask = singles.tile([128, NCH, E], f32)
                nc.vector.tensor_add(
                    mask.rearrange("p a b -> p (a b)"),
                    eq1.rearrange("p a b -> p (a b)"),
                    eq2.rearrange("p a b -> p (a b)"))
                rank = singles.tile([128, NCH, E], f32)
                with tc.tile_pool(name="rps", bufs=1, space="PSUM") as rps:
                    rk_ps = rps.tile([128, NCH * E], f32, tag="rk")
                    nc.tensor.matmul(
                        rk_ps, lstrict, mask.rearrange("p a b -> p (a b)"),
                        start=True, stop=True)
                    nc.vector.tensor_copy(
                        out=rank.rearrange("p a b -> p (a b)"), in_=rk_ps)

                # rankm = rank*eq + eq - 1  (=-1 when not routed)
                psel1 = singles.tile([128, NCH, E, CAPC], f32)
                psel2 = singles.tile([128, NCH, E, CAPC], f32)
                rkp1 = singles.tile([128, NCH, E], f32)
                nc.vector.tensor_scalar(
                    out=rkp1.rearrange("p a b -> p (a b)"),
                    in0=rank.rearrange("p a b -> p (a b)"),
                    scalar1=1.0, scalar2=None, op0=ALU.add)
                for sel, eqm, pselm in ((0, eq1, psel1), (1, eq2, psel2)):
                    rm = singles.tile([128, NCH, E], f32, name=f"rm{sel}")
                    nc.vector.tensor_mul(
                        rm.rearrange("p a b -> p (a b)"),
                        eqm.rearrange("p a b -> p (a b)"),
                        rkp1.rearrange("p a b -> p (a b)"))
                    nc.vector.tensor_scalar(
                        out=rm.rearrange("p a b -> p (a b)"),
                        in0=rm.rearrange("p a b -> p (a b)"),
                        scalar1=-1.0, scalar2=None, op0=ALU.add)
                    for k in range(NCH):
                        for e in range(E):
                            nc.vector.tensor_scalar(
                                out=pselm[:, k, e, :], in0=iota48,
                                scalar1=rm[:, k, e:e + 1], scalar2=None,
                                op0=ALU.is_equal)

                # ---------------- dispatch scatter via PE ------------------
                pairs1 = singles.tile([128, NCH, 2], f32)
                pairs2 = singles.tile([128, NCH, 2], f32)
                nc.vector.tensor_copy(out=pairs1[:, :, 0], in_=gidmat)
                nc.vector.tensor_copy(out=pairs1[:, :, 1], in_=s1_all)
                nc.vector.tensor_copy(out=pairs2[:, :, 0], in_=gidmat)
                nc.vector.tensor_copy(out=pairs2[:, :, 1], in_=s2_all)

                disp_sb = singles.tile([2, NCH, E * CAPC], f32)
                with tc.tile_pool(name="dps", bufs=1, space="PSUM") as dps:
                    disp_ps = dps.tile([2, NCH, 512], f32, tag="disp")
                    for k in range(NCH):
                        nc.tensor.matmul(
                            disp_ps[:, k, :E * CAPC], pairs1[:, k, :],
                            psel1[:, k].rearrange("p e r -> p (e r)"),
                            start=True, stop=False)
                        nc.tensor.matmul(
                            disp_ps[:, k, :E * CAPC], pairs2[:, k, :],
                            psel2[:, k].rearrange("p e r -> p (e r)"),
                            start=False, stop=True)
                    for k in range(NCH):
                        nc.vector.tensor_copy(
                            out=disp_sb[:, k, :], in_=disp_ps[:, k, :E * CAPC])

                nc.gpsimd.collective_compute(
                    "AllGather", mybir.AluOpType.bypass, replica_groups=RG,
                    ins=[w2sown_d[:].opt()], outs=[w2sall_d[:].opt()])

                # flat slot order (e, k, r): idflat[e*SEG + k*CAPC + r]
                for e in range(E):
                    nc.sync.dma_start(
                        out=idflat_d[e * SEG:(e + 1) * SEG].rearrange(
                            "(k r) -> k r", k=NCH)[None],
                        in_=disp_sb[0:1, :, e * CAPC:(e + 1) * CAPC])
                    nc.sync.dma_start(
                        out=wtflat_d[e * SEG:(e + 1) * SEG].rearrange(
                            "(k r) -> k r", k=NCH)[None],
                        in_=disp_sb[1:2, :, e * CAPC:(e + 1) * CAPC])

                # wrapped idx list for dma_gather; wt back in jtile layout
                idxf = singles.tile([128, J // 16], f32)
                idwrap = idflat_d[:].rearrange("(f p) -> p f", p=16)
                for g in range(8):
                    nc.sync.dma_start(out=idxf[16 * g:16 * (g + 1)], in_=idwrap)
                idx16 = singles.tile([128, J // 16], i16)
                nc.vector.tensor_scalar(
                    out=idxf, in0=idxf,
                    scalar1=-1.0, scalar2=0.0, op0=ALU.add, op1=ALU.max)
                nc.vector.tensor_copy(out=idx16, in_=idxf)

                wt_sb = singles.tile([128, NJ], f32)
                nc.sync.dma_start(
                    out=wt_sb, in_=wtflat_d[:].rearrange("(c p) -> p c", p=128))

                # ---------------- gather routed tokens ---------------------
                xTg_t = [singles.tile([128, KC, GI], f16, name=f"xTg{g}")
                         for g in range(NGS)]
                for g in range(NGS):
                    nc.gpsimd.dma_gather(
                        xTg_t[g][:], xown_d[:],
                        idx16[:, g * (GI // 16):(g + 1) * (GI // 16)],
                        GI, GI, D, transpose=True)

                # b2sum per expert (b2 is usually zero; cheap generic path)
                b2s_bc = singles.tile([128, E], f32)
                with tc.tile_pool(name="b2p", bufs=1) as b2p:
                    b2t = b2p.tile([E, D], f32, tag="b2")
                    nc.sync.dma_start(out=b2t, in_=b2_d[:])
                    b2s_col = b2p.tile([E, 1], f32, tag="b2s")
                    nc.vector.reduce_sum(b2s_col, b2t, axis=AX.X)
                    nc.sync.dma_start(out=b2s8_d[:, None], in_=b2s_col)
                    nc.gpsimd.dma_start(
                        out=b2s_bc,
                        in_=bass.AP(tensor=b2s8_d.tensor, offset=b2s8_d.offset,
                                    ap=[[0, 128]] + [list(a) for a in b2s8_d.ap]))
                if has_b1:
                    b1_sb = singles.tile([1, E, H], f32)
                    nc.sync.dma_start(out=b1_sb, in_=b1_d[None])

                # ---------------- main sparse GEMM (expert-major) ----------
                phat = singles.tile([128, NJ], f32)
                with tc.tile_pool(name="w1p", bufs=2) as w1p, \
                     tc.tile_pool(name="w2b", bufs=2) as w2b, \
                     tc.tile_pool(name="gp", bufs=2) as gp, \
                     tc.tile_pool(name="psm", bufs=2, space="PSUM") as psm:
                    for e in range(E):
                        w1t = w1p.tile([128, KC, H], f16, tag="w1")
                        nc.sync.dma_start(
                            out=w1t, in_=w1_d[e].rearrange("(k p) h -> p k h", p=128))
                        w2e = w2sall_d[e]
                        w2s_bc = w2b.tile([128, H], f32, tag="w2s")
                        nc.gpsimd.dma_start(
                            out=w2s_bc,
                            in_=bass.AP(tensor=w2e.tensor, offset=w2e.offset,
                                        ap=[[0, 128]] + [list(a) for a in w2e.ap]))
                        for j in range(JPE):
                            jt = e * JPE + j
                            xTg = xTg_t[jt // (GI // 128)]
                            jl = jt % (GI // 128)
                            hp = psm.tile([128, H], f32, tag="hp")
                            for k in range(KC):
                                lhsT = xTg[:, k, jl * 128:(jl + 1) * 128]
                                for n in range(NH):
                                    nc.tensor.matmul(
                                        hp[:, n * 512:(n + 1) * 512], lhsT,
                                        w1t[:, k, n * 512:(n + 1) * 512],
                                        start=(k == 0),
                                        stop=(k == KC - 1 and not has_b1))
                            if has_b1:
                                for n in range(NH):
                                    nc.tensor.matmul(
                                        hp[:, n * 512:(n + 1) * 512], ones_row,
                                        b1_sb[:, e, n * 512:(n + 1) * 512],
                                        start=False, stop=True)
                            g_sb = gp.tile([128, H], f32, tag="g")
                            nc.scalar.activation(out=g_sb, in_=hp, func=ACT.Gelu)
                            nc.vector.scalar_tensor_tensor(
                                out=g_sb, in0=g_sb, scalar=1.0, in1=w2s_bc,
                                op0=ALU.mult, op1=ALU.mult,
                                accum_out=phat[:, jt:jt + 1])

                # contribs = (phat + b2sum[e]) * wt, in slot order
                cont_sb = singles.tile([128, NJ], f32)
                for e in range(E):
                    nc.vector.tensor_scalar(
                        out=cont_sb[:, e * JPE:(e + 1) * JPE],
                        in0=phat[:, e * JPE:(e + 1) * JPE],
                        scalar1=b2s_bc[:, e:e + 1], scalar2=None, op0=ALU.add)
                nc.vector.tensor_mul(cont_sb, cont_sb, wt_sb)
                nc.sync.dma_start(
                    out=cont_d[:].rearrange("(c p) -> p c", p=128), in_=cont_sb)

                # ---------------- combine + log_softmax --------------------
                y_sb = singles.tile([128, NCH], f32)
                with tc.tile_pool(name="cmb", bufs=4) as cmb, \
                     tc.tile_pool(name="fin", bufs=2) as fin, \
                     tc.tile_pool(name="psf", bufs=1, space="PSUM") as psf:
                    for k in range(NCH):
                        rrow = cmb.tile([1, E, CAPC], f32, tag="rrow")
                        nc.sync.dma_start(
                            out=rrow,
                            in_=cont_d[:].rearrange(
                                "(e k r) -> e k r", e=E, k=NCH)[None, :, k, :])
                        rbc = cmb.tile([128, E * CAPC], f32, tag="rbc")
                        nc.gpsimd.partition_broadcast(
                            rbc, rrow.rearrange("p a b -> p (a b)"))
                        y1 = cmb.tile([128, 1], f32, tag="y1")
                        y2 = cmb.tile([128, 1], f32, tag="y2")
                        sc1 = cmb.tile([128, E * CAPC], f32, tag="sc1")
                        nc.vector.scalar_tensor_tensor(
                            out=sc1, in0=psel1[:, k].rearrange("p e r -> p (e r)"),
                            scalar=1.0, in1=rbc, op0=ALU.mult, op1=ALU.mult,
                            accum_out=y1)
                        sc2 = cmb.tile([128, E * CAPC], f32, tag="sc2")
                        nc.vector.scalar_tensor_tensor(
                            out=sc2, in0=psel2[:, k].rearrange("p e r -> p (e r)"),
                            scalar=1.0, in1=rbc, op0=ALU.mult, op1=ALU.mult,
                            accum_out=y2)
                        nc.vector.tensor_add(y_sb[:, k:k + 1], y1, y2)

                    _tail(nc, tc, psf, fin, out_d, ident, ones_col, ones_row,
                          y_sb, f32, ALU, ACT, AX)

    nc.compile()
    return nc


def get_nc(has_b1: bool):
    key = (has_b1,)
    if key not in _CACHE:
        _CACHE[key] = _build(has_b1)
    return _CACHE[key]


def make_in_maps(x, gate_w, gate_b, w1, b1, w2, b2):
    f = np.float32
    x = np.asarray(x, f)
    gate_w = np.asarray(gate_w, f)
    gw16 = gate_w.astype(np.float16)
    gwr16 = (gate_w - gw16.astype(f)).astype(np.float16)
    gwcat = np.concatenate([gw16, gwr16], axis=1)  # [D, 16]

    gid = (np.arange(128, dtype=f)[:, None]
           + (np.arange(NCH, dtype=f) * 128)[None, :] + 1.0)
    common = {
        "gwcat16": np.ascontiguousarray(gwcat),
        "gate_b": np.ascontiguousarray(gate_b, dtype=f),
        "w1all": np.ascontiguousarray(np.asarray(w1, f)).astype(np.float16),
        "b1all": np.ascontiguousarray(b1, dtype=f),
        "b2all": np.ascontiguousarray(b2, dtype=f),
        "ident128": np.eye(128, dtype=f),
        "lstrict": np.ascontiguousarray(np.tril(np.ones((128, 128), f), -1).T),
        "iota48": np.broadcast_to(np.arange(CAPC, dtype=f), (128, CAPC)).copy(),
        "gidmat": np.ascontiguousarray(gid),
    }
    maps = []
    for c in range(B):
        xc = x[c]                      # [S, D]
        xc16 = xc.astype(np.float16)
        xr16 = (xc - xc16.astype(f)).astype(np.float16)
        maps.append({
            "xT16": np.ascontiguousarray(xc16.T),
            "xTr16": np.ascontiguousarray(xr16.T),
            "x_own16": np.ascontiguousarray(xc16),
            "w2c": np.ascontiguousarray(w2[c], dtype=f),
            **common,
        })
    return maps


def kernel(x, gate_w, gate_b, w1, b1, w2, b2):
    from concourse.bass_utils import run_bass_kernel_spmd

    has_b1 = bool(np.any(np.asarray(b1)))
    nc = get_nc(has_b1)
    in_maps = make_in_maps(x, gate_w, gate_b, w1, b1, w2, b2)
    res = run_bass_kernel_spmd(nc, in_maps, core_ids=list(range(B)))
    return np.stack([res.results[c]["out"] for c in range(B)]).astype(np.float32)


import concourse.bass as bass  # noqa: E402  (used by _build at call time)
